# revision 37
# baseline (speedup 1.0000x reference)
"""Trainium2 Bass kernel for nn_ContextModel_85993835200994 — fp8 DoubleRow.

PixelCNN-style context model (see reference):
  out = round(x); masked 5x5 conv (12 taps) 192->384; h=concat(conv,phi) 768
  h1 = leaky(h@w1+b1) 640; h2 = leaky(h1@w2+b2) 640
  cond = h2@w3+b3 = [mean|scale]; lik = Phi((v+.5)/s)-Phi((v-.5)/s)

All matmuls run as fp8e4 DoubleRow (K=256 per matmul, 0.5 cyc/row) with
error compensation: weights are pre-scaled by a per-tensor 2^k (avoids the
e4m3 subnormal floor) and split hi+lo; activations are evacuated to fp16
then split hi+lo on-device. Each layer computes Wh@(Hh+Hl) + Wl@Hh
(~8 effective mantissa bits). x=round(x) is exact in fp8, so the conv
needs only the weight split, done as one broadcast-pair DoubleRow per tap.
The 5-k-tile layers pair the odd k-tile's hi/lo terms in one broadcast DR.
mlp3 runs as 3 full-width M=128 groups (columns repacked as
[mean128:192|scale128:192][scale0:128][mean0:128]); the mixed group's
scale half sits at PSUM partitions 64:128 and is realigned to base 0 by
its ACT Abs evac. The likelihood runs in "scaled units" (PSUM carries
2^k3 * cond; the 2^k3 cancels between the mean and scale halves), fp16
elementwise with a single fused Erf over [em|ep], and the final 0.5x is
folded into the host-side gather.

Emission is software-pipelined at depth 4 (mlp3 leads each iteration so
its likelihood ops take engine-queue priority and its PSUM banks free
fast). Conv inputs arrive as ONE DMA per chunk from a host-packed
3-plane tensor (plane 1/2 bake the dual-tap row/col shifts); weights ship
as 5 merged DMAs. The two final half-chunks spread their evacuations
across ACT/DVE/Pool to shorten the drain's serial chains.

Distribution: data-parallel over batch x image-half -> 8 cores, each
computing a [192, 64, 128] output slice (mode-A conv needs 2 halo rows
above only).
"""

import numpy as np
import ml_dtypes

import concourse.bass as bass
import concourse.mybir as mybir
import concourse.tile as tile
from concourse import bacc
from concourse.bass_utils import run_bass_kernel_spmd

F32 = mybir.dt.float32
F16 = mybir.dt.float16
F8 = mybir.dt.float8e4
AF = mybir.ActivationFunctionType
ALU = mybir.AluOpType
DR = mybir.MatmulPerfMode.DoubleRow
F8NP = ml_dtypes.float8_e4m3
E4MAX = 224.0

C_LAT = 192
C_PHI = 384
HID = 640
B, H, W = 4, 128, 128
N_CORES = 8
ROWS = 64
CHUNKS = [(i * 4, 4) for i in range(15)] + [(60, 2), (62, 2)]
NCH = len(CHUNKS)
XR_H = ROWS + 3
XR_W = W + 6
SQRT2 = 1.4142135623730951

TAPS = [(dy, dx) for dy in (-2, -1) for dx in (-2, -1, 0, 1, 2)] + \
       [(0, -2), (0, -1)]
NT = len(TAPS)
NTK = 18          # conv k-tiles: 12 ch-lo taps + 6 dual-tap ch-hi

TRACE = False
LAST_RESULT = None
_CACHE = {}


def _build(kc, k1, k2, k3):
    nc = bacc.Bacc("TRN2", target_bir_lowering=False, debug=False)

    xa_d = nc.dram_tensor("xa", [128, 3, XR_H, XR_W], F8, kind="ExternalInput").ap()
    phi_d = nc.dram_tensor("phi", [128, 2, 3, ROWS, W], F8, kind="ExternalInput").ap()
    wc_d = nc.dram_tensor("wc", [128, NTK, 2, C_PHI], F8, kind="ExternalInput").ap()
    w1_d = nc.dram_tensor("w1", [128, 6, 2, HID], F8, kind="ExternalInput").ap()
    w2_d = nc.dram_tensor("w2", [128, 5, 2, HID], F8, kind="ExternalInput").ap()
    w3_d = nc.dram_tensor("w3", [128, 5, 2, 2 * C_LAT], F8, kind="ExternalInput").ap()
    bias_d = nc.dram_tensor("bias", [128, 21], F32, kind="ExternalInput").ap()
    lik_d = nc.dram_tensor("lik", [C_LAT, ROWS, W], F16, kind="ExternalOutput").ap()

    S = float(2.0 ** k3)
    CLAMP = float(0.11 * SQRT2 * S)

    with tile.TileContext(nc) as tc:
        with tc.tile_pool(name="const", bufs=1) as cpool, \
             tc.tile_pool(name="rp", bufs=4) as rpool, \
             tc.tile_pool(name="hp", bufs=2) as hpool, \
             tc.tile_pool(name="tp", bufs=8) as tpool, \
             tc.tile_pool(name="ps", bufs=8, space="PSUM") as pspool:

            wc_s = cpool.tile([128, NTK, 2, C_PHI], F8, tag="wc")
            w1_s = cpool.tile([128, 6, 2, HID], F8, tag="w1")
            w2_s = cpool.tile([128, 5, 2, HID], F8, tag="w2")
            w3_s = cpool.tile([128, 5, 2, 2 * C_LAT], F8, tag="w3")
            bs_s = cpool.tile([128, 21], F32, tag="bs")
            bc_s = bs_s[:, 0:3]
            b1_s = bs_s[:, 3:8]
            b2_s = bs_s[:, 8:13]
            b3_s = bs_s[:, 13:21]

            st = {}  # per-chunk tile state

            # split-op engine rotation (13 splits/chunk): conv splits (0-2)
            # stay off DVE entirely (ACT copy + Pool subtract) so the DVE
            # queue reaches the m2-gating m1 splits (3-7) earlier
            v, g, a = nc.vector, nc.gpsimd, nc.scalar
            hi_cycle = [a, a, a,  v, v, v, v, v,  v, g, v, g, v]
            lo_cycle = [g, g, g,  v, g, v, g, v,  g, v, g, v, g]

            def split(h16v, hhv, hlv, idx):
                if hi_cycle[idx] is a:
                    a.activation(hhv, h16v, AF.Copy)
                else:
                    hi_cycle[idx].tensor_copy(hhv, h16v)
                lo_cycle[idx].tensor_tensor(hlv, h16v, hhv, ALU.subtract)

            def evac(p_v, h16v, bcol, kk, m, ci, lbl, prelu):
                """psum -> fp16 h. Wide (drain) chunks spread the work:
                ACT for m 0/3, DVE/Pool two-op affine(+leaky) otherwise."""
                func = AF.Prelu if prelu else AF.Identity
                sc_ = float(2.0 ** -kk)
                if ci < NCH - 2 or m in (0, 2, 3):
                    nc.scalar.activation(h16v, p_v, func, bias=bcol,
                                         scale=sc_,
                                         **({"alpha": 0.01} if prelu else {}))
                    return
                # DVE reads PSUM for the affine; Pool (no PSUM access) does
                # the SBUF-to-SBUF leaky
                if not prelu:
                    v.tensor_scalar(h16v, p_v, sc_, bcol, ALU.mult, ALU.add)
                    return
                aff = tpool.tile([128, 512], F16, tag="ev",
                                 name=f"af{lbl}_{m}_{ci}")
                n = h16v.shape[-1]
                v.tensor_scalar(aff[:, :n], p_v, sc_, bcol, ALU.mult, ALU.add)
                v.scalar_tensor_tensor(h16v, aff[:, :n], 0.01, aff[:, :n],
                                       ALU.mult, ALU.max)

            def conv_emit(ci):
                y0, rows = CHUNKS[ci]
                N = rows * 128
                nr = rows + 2
                XA = rpool.tile([128, 3, 6, W + 4], F8, tag="XA")
                if ci == 0:
                    # stage the first matmul's critical path in tiny DMAs
                    nc.sync.dma_start(wc_s[:, 0:1], wc_d[:, 0:1])
                    nc.sync.dma_start(XA[:, 0, 0:nr], xa_d[:, 0, y0:y0 + nr, 0:W + 4])
                    nc.sync.dma_start(wc_s[:, 1:3], wc_d[:, 1:3])
                    nc.sync.dma_start(XA[:, 1, 0:nr], xa_d[:, 1, y0:y0 + nr, 0:W + 4])
                    nc.sync.dma_start(XA[:, 2, 0:nr], xa_d[:, 2, y0:y0 + nr, 0:W + 4])
                    nc.sync.dma_start(wc_s[:, 3:9], wc_d[:, 3:9])
                elif nr == 6:
                    nc.sync.dma_start(XA[:], xa_d[:, :, y0:y0 + nr, 0:W + 4])
                else:
                    for pl in range(3):
                        nc.sync.dma_start(XA[:, pl, 0:nr],
                                          xa_d[:, pl, y0:y0 + nr, 0:W + 4])
                HC = hpool.tile([128, 2, 6, rows, W], F8, tag="HC")
                nc.sync.dma_start(HC[:, 0, 0:3], phi_d[:, 0, :, y0:y0 + rows, :])
                nc.sync.dma_start(HC[:, 1, 0:3], phi_d[:, 1, :, y0:y0 + rows, :])
                if ci == 0:
                    nc.sync.dma_start(wc_s[:, 9:NTK], wc_d[:, 9:NTK])
                    nc.sync.dma_start(bs_s[:], bias_d)

                pc = [pspool.tile([128, 512], F32, tag="ps", name=f"pc{m}_{ci}")
                      for m in range(3)]
                for t in range(NTK):
                    if t < 12:
                        dy, dx = TAPS[t]
                        src = XA[:, 0]
                    elif t < 17:
                        dy, dx = TAPS[t - 12]
                        src = XA[:, 1]
                    else:
                        dy, dx = TAPS[10]
                        src = XA[:, 2]
                    win = src[:, 2 + dy:2 + rows + dy, 2 + dx:2 + dx + W]
                    winb = win.unsqueeze(1).broadcast_to([128, 2, rows, W])
                    for m in range(3):
                        ms = slice(m * 128, (m + 1) * 128)
                        nc.tensor.matmul(pc[m][:, :N], wc_s[:, t, :, ms], winb,
                                         start=(t == 0), stop=(t == NTK - 1),
                                         perf_mode=DR)
                if ci == 0:
                    # k-tiles 0:2 first: mlp1(0)'s first DRs need only these
                    nc.sync.dma_start(w1_s[:, 0:2], w1_d[:, 0:2])
                    nc.sync.dma_start(w1_s[:, 2:6], w1_d[:, 2:6])

                hc16 = hpool.tile([128, 3, 512], F16, tag="hc16")
                for m in range(3):
                    evac(pc[m][:, :N], hc16[:, m, :N], bc_s[:, m:m + 1],
                         kc, m, ci, "c", False)
                    split(hc16[:, m, :N],
                          HC[:, 0, 3 + m].rearrange("p r w -> p (r w)"),
                          HC[:, 1, 3 + m].rearrange("p r w -> p (r w)"), m)
                st[ci] = dict(XA=XA, HC=HC, rows=rows, y0=y0, N=N)

            def _5k_seq(Wp, Ah, Al, ms, n):
                """8-DR term sequence for one m-group, ordered by how late its
                input split lands. Wp layout [128, ktile, 2(hi/lo), D]."""
                a4h = Ah[:, 4, :n].unsqueeze(1).broadcast_to([128, 2, n])
                a4l = Al[:, 4, :n].unsqueeze(1).broadcast_to([128, 2, n])
                return [
                    (Wp[:, 0:2, 0, ms], Ah[:, 0:2, :n]),
                    (Wp[:, 0:2, 1, ms], Ah[:, 0:2, :n]),
                    (Wp[:, 0:2, 0, ms], Al[:, 0:2, :n]),
                    (Wp[:, 2:4, 0, ms], Ah[:, 2:4, :n]),
                    (Wp[:, 2:4, 1, ms], Ah[:, 2:4, :n]),
                    (Wp[:, 2:4, 0, ms], Al[:, 2:4, :n]),
                    (Wp[:, 4, :, ms], a4h),
                    (Wp[:, 4, :, ms], a4l),
                ]

            def mlp_5k(psum_t, Wp, Ah, Al, ms, n):
                """group-serial: all 8 DRs of one m-group back-to-back."""
                seq = _5k_seq(Wp, Ah, Al, ms, n)
                for i, (wv, av) in enumerate(seq):
                    nc.tensor.matmul(psum_t, wv, av, start=(i == 0),
                                     stop=(i == len(seq) - 1), perf_mode=DR)

            def mlp_5k_phased(psums, Wp, Ah, Al, n):
                """phase-major across all m-groups: every group's term-j DRs
                issue before any group's term-j+1, so split production stays
                ahead of consumption."""
                seqs = [_5k_seq(Wp, Ah, Al, slice(m * 128, (m + 1) * 128), n)
                        for m in range(len(psums))]
                for ph in range(8):
                    for m, pt in enumerate(psums):
                        wv, av = seqs[m][ph]
                        nc.tensor.matmul(pt, wv, av, start=(ph == 0),
                                         stop=(ph == 7), perf_mode=DR)

            def mlp1_emit(ci):
                s = st[ci]
                N = s["N"]
                HCv = s["HC"]
                Ah, Al = HCv[:, 0], HCv[:, 1]
                p1 = [pspool.tile([128, 512], F32, tag="ps", name=f"p1_{m}_{ci}")
                      for m in range(5)]
                for m in range(5):
                    ms = slice(m * 128, (m + 1) * 128)
                    seq = []
                    for j in range(3):
                        seq += [(0, Ah, j), (1, Ah, j), (0, Al, j)]
                    for i, (hl, hs, j) in enumerate(seq):
                        nc.tensor.matmul(p1[m][:, :N],
                                         w1_s[:, 2 * j:2 * j + 2, hl, ms],
                                         hs[:, 2 * j:2 * j + 2],
                                         start=(i == 0), stop=(i == len(seq) - 1),
                                         perf_mode=DR)
                h1_16 = hpool.tile([128, 5, 512], F16, tag="h1_16")
                H1h = hpool.tile([128, 5, 512], F8, tag="H1h")
                H1l = hpool.tile([128, 5, 512], F8, tag="H1l")
                for m in range(5):
                    evac(p1[m][:, :N], h1_16[:, m, :N], b1_s[:, m:m + 1],
                         k1, m, ci, "1", True)
                    split(h1_16[:, m, :N], H1h[:, m, :N], H1l[:, m, :N], 3 + m)
                if ci == 0:
                    nc.sync.dma_start(w2_s[:, 0:2], w2_d[:, 0:2])
                    nc.sync.dma_start(w2_s[:, 2:5], w2_d[:, 2:5])
                s["H1h"], s["H1l"] = H1h, H1l

            def mlp2_emit(ci):
                s = st[ci]
                N = s["N"]
                H1h, H1l = s["H1h"], s["H1l"]
                p2 = [pspool.tile([128, 512], F32, tag="ps", name=f"p2_{m}_{ci}")
                      for m in range(5)]
                for m in range(5):
                    ms = slice(m * 128, (m + 1) * 128)
                    mlp_5k(p2[m][:, :N], w2_s, H1h, H1l, ms, N)
                h2_16 = hpool.tile([128, 5, 512], F16, tag="h2_16")
                H2h = hpool.tile([128, 5, 512], F8, tag="H2h")
                H2l = hpool.tile([128, 5, 512], F8, tag="H2l")
                for m in range(5):
                    evac(p2[m][:, :N], h2_16[:, m, :N], b2_s[:, m:m + 1],
                         k2, m, ci, "2", True)
                    split(h2_16[:, m, :N], H2h[:, m, :N], H2l[:, m, :N], 8 + m)
                if ci == 0:
                    nc.sync.dma_start(w3_s[:], w3_d)
                s["H2h"], s["H2l"] = H2h, H2l  # noqa

            def lik_emit(ci, s, g, P, pm, psc, Rg, cpc, cmc, bsc, bsc_base):
                """Likelihood chain for one channel group. `psc` may sit at a
                nonzero PSUM base partition; the ACT Abs evac realigns it to
                base 0 (PSUM in + SB out cross-base is legal)."""
                rows, y0, N = s["rows"], s["y0"], s["N"]
                tg = f"t{g}"
                Rc = Rg[0:P, 2:2 + rows, 2:2 + W]
                # scale chain first: abs -> max -> recip (off mean critical path)
                sabs = tpool.tile([P, 512], F32, tag=tg, name=f"sa{g}_{ci}")
                nc.scalar.activation(sabs[:, :N], psc[:, :N], AF.Abs,
                                     bias=b3_s[bsc_base:bsc_base + P,
                                               bsc:bsc + 1])
                sc = tpool.tile([P, 512], F32, tag=tg, name=f"sc{g}_{ci}")
                nc.gpsimd.tensor_scalar_max(sc[:, :N], sabs[:, :N], CLAMP)
                rq = tpool.tile([P, 512], F32, tag=tg, name=f"rq{g}_{ci}")
                nc.vector.reciprocal_approx_fast(out=rq[:, :N], in_=sc[:, :N])
                tt = tpool.tile([P, 512], F16, tag=tg, name=f"tt{g}_{ci}")
                nc.vector.scalar_tensor_tensor(
                    tt[:, :N], Rc, -S, pm[:, :N], ALU.mult, ALU.add)
                # [em | ep] packed so one Erf covers both halves
                E = tpool.tile([P, 2, 512], F16, tag=tg, name=f"E{g}_{ci}")
                nc.vector.scalar_tensor_tensor(
                    E[:, 0, :N], tt[:, :N], b3_s[0:P, cpc:cpc + 1], rq[:, :N],
                    ALU.add, ALU.mult)
                nc.vector.scalar_tensor_tensor(
                    E[:, 1, :N], tt[:, :N], b3_s[0:P, cmc:cmc + 1], rq[:, :N],
                    ALU.add, ALU.mult)
                E2 = tpool.tile([P, 2, 512], F16, tag=tg, name=f"F{g}_{ci}")
                nc.scalar.activation(E2[:, :, :N], E[:, :, :N], AF.Erf)
                dd = tpool.tile([P, 512], F16, tag=tg, name=f"dd{g}_{ci}")
                ddeng = nc.vector if ci >= NCH - 1 else nc.gpsimd
                ddeng.tensor_tensor(dd[:, :N], E2[:, 0, :N], E2[:, 1, :N],
                                    ALU.subtract)
                ch0 = 0 if g == 0 else 128
                nc.sync.dma_start(lik_d[ch0:ch0 + P, y0:y0 + rows, :],
                                  dd[:, :N])

            def mlp3_emit(ci):
                s = st.pop(ci)
                N = s["N"]
                H2h, H2l, XA = s["H2h"], s["H2l"], s["XA"]
                # 3 M=128 groups, packed [mean128:192|scale128:192],
                # [scale0:128], [mean0:128]; the mixed group goes first so its
                # (small) likelihood chain overlaps the remaining matmuls, and
                # the g0 scale chain overlaps the g0 mean matmuls.
                p3 = []
                for mi in range(3):
                    pt = pspool.tile([128, 512], F32, tag="ps", name=f"p3_{mi}_{ci}")
                    mlp_5k(pt[:, :N], w3_s, H2h, H2l,
                           slice(mi * 128, (mi + 1) * 128), N)
                    p3.append(pt)
                    if mi == 0:
                        lik_emit(ci, s, 1, 64, pt[0:64], pt[64:128], XA[:, 1],
                                 4, 5, 6, 64)
                lik_emit(ci, s, 0, 128, p3[2], p3[1], XA[:, 0], 0, 1, 2, 0)

            # depth-4 software pipeline. mlp3 leads each iteration so its
            # likelihood chain ops sit at the head of the engine queues (p3
            # PSUM banks free fast) and every 8-back psum-pool pairing lands
            # on an ACT-evac-freed tile.
            for i in range(NCH + 3):
                if i < NCH:
                    # steady state: m3 first (lik ops take queue priority,
                    # p3 banks free fast)
                    if 3 <= i:
                        mlp3_emit(i - 3)
                    conv_emit(i)
                    if 1 <= i:
                        mlp1_emit(i - 1)
                    if 2 <= i:
                        mlp2_emit(i - 2)
                else:
                    # drain: evacs gate the remaining PE work — emit them
                    # ahead of the likelihood chains in the engine FIFOs
                    if i <= NCH:
                        mlp1_emit(i - 1)
                    if i <= NCH + 1:
                        mlp2_emit(i - 2)
                    mlp3_emit(i - 3)

    nc.compile()
    return nc


def _wsplit(wt):
    """per-tensor 2^k scaling + e4m3 hi/lo split. Returns (hi, lo, k)."""
    k = int(np.floor(np.log2(E4MAX / np.abs(wt).max())))
    ws = (wt * (2.0 ** k)).astype(np.float32)
    hi = ws.astype(F8NP)
    lo = (ws - hi.astype(np.float32)).astype(F8NP)
    return hi, lo, k


def _host_pack(mask_w, mask_b, w1, b1, w2, b2, w3, b3):
    wc = np.empty((C_LAT, NT, C_PHI), np.float32)
    for t, (dy, dx) in enumerate(TAPS):
        wc[:, t, :] = mask_w[:, :, dy + 2, dx + 2].T
    wcp = np.empty((128, NTK, C_PHI), np.float32)
    wcp[:, :12] = wc[:128]
    for j in range(6):
        ta, tb = (j, 5 + j) if j < 5 else (10, 11)
        wcp[0:64, 12 + j] = wc[128:, ta]
        wcp[64:128, 12 + j] = wc[128:, tb]
    wch, wcl, kc = _wsplit(wcp)
    wc8 = np.ascontiguousarray(np.stack([wch, wcl], axis=2))

    # w1 rows reordered: slots 0-2 = phi (rows 384:768), 3-5 = conv (0:384)
    w1r = np.concatenate([w1[C_PHI:], w1[:C_PHI]], axis=0)
    w1p = np.ascontiguousarray(w1r.reshape(6, 128, HID).transpose(1, 0, 2))
    w1h, w1l, k1 = _wsplit(w1p)
    w1pk = np.ascontiguousarray(np.stack([w1h, w1l], axis=2))

    w2p = np.ascontiguousarray(w2.reshape(5, 128, HID).transpose(1, 0, 2))
    w2h, w2l, k2 = _wsplit(w2p)
    w2pk = np.ascontiguousarray(np.stack([w2h, w2l], axis=2))

    w3m = w3.copy()
    w3m[:, C_LAT:] *= SQRT2
    # column order = [mean128:192 | scale128:192] [scale0:128] [mean0:128]
    # so mlp3 runs as 3 full-width M=128 groups (mixed group first)
    perm = np.concatenate([np.arange(128, 192), np.arange(320, 384),
                           np.arange(192, 320), np.arange(0, 128)])
    w3m = np.ascontiguousarray(w3m[:, perm])
    w3p = np.ascontiguousarray(w3m.reshape(5, 128, 2 * C_LAT).transpose(1, 0, 2))
    w3h, w3l, k3 = _wsplit(w3p)
    w3pk = np.ascontiguousarray(np.stack([w3h, w3l], axis=2))

    S = 2.0 ** k3
    b3pk = np.zeros((128, 8), np.float32)
    b3pk[:, 0] = S * (b3[0:128] + 0.5)
    b3pk[:, 1] = S * (b3[0:128] - 0.5)
    b3pk[:, 2] = S * SQRT2 * b3[192:320]
    b3pk[:64, 4] = S * (b3[128:192] + 0.5)
    b3pk[:64, 5] = S * (b3[128:192] - 0.5)
    # scale bias for ch 128:192 duplicated at both partition halves so the
    # ACT Abs bias is right under either base-alignment convention
    b3pk[:64, 6] = S * SQRT2 * b3[320:384]
    b3pk[64:, 6] = S * SQRT2 * b3[320:384]

    bias_pk = np.ascontiguousarray(np.concatenate(
        [mask_b.reshape(3, 128).T, b1.reshape(5, 128).T,
         b2.reshape(5, 128).T, b3pk], axis=1)).astype(np.float32)

    weights = {
        "wc": wc8,
        "w1": w1pk,
        "w2": w2pk,
        "w3": w3pk,
        "bias": bias_pk,
    }
    return weights, (kc, k1, k2, k3)


def kernel(x, phi, mask_w, mask_b, w1, b1, w2, b2, w3, b3):
    global LAST_RESULT
    x = np.asarray(x, dtype=np.float32)
    phi = np.asarray(phi, dtype=np.float32)
    weights, ks = _host_pack(
        np.asarray(mask_w, np.float32), np.asarray(mask_b, np.float32),
        np.asarray(w1, np.float32), np.asarray(b1, np.float32),
        np.asarray(w2, np.float32), np.asarray(b2, np.float32),
        np.asarray(w3, np.float32), np.asarray(b3, np.float32))

    R = np.round(x)
    R8 = R.astype(F8NP)
    phih = phi.astype(F8NP)
    phil = (phi - phih.astype(np.float32)).astype(F8NP)

    key = ("nc",) + ks
    if key not in _CACHE:
        _CACHE[key] = _build(*ks)
        _CACHE["nc"] = _CACHE[key]
    nc = _CACHE[key]

    in_maps = []
    for c in range(N_CORES):
        b, r0 = c // 2, (c % 2) * ROWS
        xr_c = np.zeros((C_LAT, XR_H, XR_W), F8NP)
        lo = max(r0 - 2, 0)
        hi = min(r0 + ROWS + 1, H)
        xr_c[:, 2 - (r0 - lo):2 - (r0 - lo) + (hi - lo), 2:2 + W] = R8[b, :, lo:hi, :]
        # 3 planes: [ch0:128] | [ch128:192; +1row] | [ch128:192; +1col]
        xa_c = np.zeros((128, 3, XR_H, XR_W), F8NP)
        xa_c[:, 0] = xr_c[0:128]
        xa_c[0:64, 1] = xr_c[128:192]
        xa_c[64:128, 1, 0:XR_H - 1] = xr_c[128:192, 1:XR_H]
        xa_c[0:64, 2] = xr_c[128:192]
        xa_c[64:128, 2, :, 0:XR_W - 1] = xr_c[128:192, :, 1:XR_W]
        # phi packed [128, 2, 3, ROWS, W]: hi/lo, partition-major k-tiles
        ph_c = np.ascontiguousarray(
            phih[b, :, r0:r0 + ROWS, :].reshape(3, 128, ROWS, W)
            .transpose(1, 0, 2, 3))
        pl_c = np.ascontiguousarray(
            phil[b, :, r0:r0 + ROWS, :].reshape(3, 128, ROWS, W)
            .transpose(1, 0, 2, 3))
        phi_c = np.ascontiguousarray(np.stack([ph_c, pl_c], axis=1))
        in_maps.append({"xa": xa_c, "phi": phi_c, **weights})

    res = run_bass_kernel_spmd(nc, in_maps, core_ids=list(range(N_CORES)),
                               trace=TRACE)
    LAST_RESULT = res

    lik = np.empty((B, C_LAT, H, W), np.float32)
    for c in range(N_CORES):
        b, r0 = c // 2, (c % 2) * ROWS
        lik[b, :, r0:r0 + ROWS, :] = \
            np.asarray(res.results[c]["lik"], np.float32) * 0.5
    return R, lik


# revision 42
# speedup vs baseline: 1.0848x; 1.0848x over previous
"""Trainium2 Bass kernel for nn_ContextModel_85993835200994 — fp8 DoubleRow.

PixelCNN-style context model (see reference):
  out = round(x); masked 5x5 conv (12 taps) 192->384; h=concat(conv,phi) 768
  h1 = leaky(h@w1+b1) 640; h2 = leaky(h1@w2+b2) 640
  cond = h2@w3+b3 = [mean|scale]; lik = Phi((v+.5)/s)-Phi((v-.5)/s)

All matmuls run as fp8e4 DoubleRow (K=256 per matmul, 0.5 cyc/row) with
error compensation: weights are pre-scaled by a per-tensor 2^k (avoids the
e4m3 subnormal floor) and split hi+lo; activations are evacuated to fp16
then split hi+lo on-device. Each layer computes Wh@(Hh+Hl) + Wl@Hh
(~8 effective mantissa bits). x=round(x) is exact in fp8, so the conv
needs only the weight split, done as one broadcast-pair DoubleRow per tap.
The 5-k-tile layers pair the odd k-tile's hi/lo terms in one broadcast DR.
mlp3 runs as 3 full-width M=128 groups (columns repacked as
[mean128:192|scale128:192][scale0:128][mean0:128]); the mixed group's
scale half sits at PSUM partitions 64:128 and is realigned to base 0 by
its ACT Abs evac. The likelihood runs in "scaled units" (PSUM carries
2^k3 * cond; the 2^k3 cancels between the mean and scale halves), fp16
elementwise with a single fused Erf over [em|ep], and the final 0.5x is
folded into the host-side gather.

Emission is software-pipelined at depth 4 (mlp3 leads each iteration so
its likelihood ops take engine-queue priority and its PSUM banks free
fast). Conv inputs arrive as ONE DMA per chunk from a host-packed
3-plane tensor (plane 1/2 bake the dual-tap row/col shifts); weights ship
as 5 merged DMAs. The two final half-chunks spread their evacuations
across ACT/DVE/Pool to shorten the drain's serial chains.

Distribution: data-parallel over batch x image-half -> 8 cores, each
computing a [192, 64, 128] output slice (mode-A conv needs 2 halo rows
above only).
"""

import numpy as np
import ml_dtypes

import concourse.bass as bass
import concourse.mybir as mybir
import concourse.tile as tile
from concourse import bacc
from concourse.bass_utils import run_bass_kernel_spmd

F32 = mybir.dt.float32
F16 = mybir.dt.float16
F8 = mybir.dt.float8e4
AF = mybir.ActivationFunctionType
ALU = mybir.AluOpType
DR = mybir.MatmulPerfMode.DoubleRow
F8NP = ml_dtypes.float8_e4m3
E4MAX = 224.0

C_LAT = 192
C_PHI = 384
HID = 640
B, H, W = 4, 128, 128
N_CORES = 8
ROWS = 64
CHUNKS = [(i * 4, 4) for i in range(15)] + [(60, 2), (62, 2)]
NCH = len(CHUNKS)
XR_H = ROWS + 3
XR_W = W + 6
SQRT2 = 1.4142135623730951

TAPS = [(dy, dx) for dy in (-2, -1) for dx in (-2, -1, 0, 1, 2)] + \
       [(0, -2), (0, -1)]
NT = len(TAPS)
NTK = 18          # conv k-tiles: 12 ch-lo taps + 6 dual-tap ch-hi

TRACE = False
LAST_RESULT = None
_CACHE = {}


def _build(kc, k1, k2, k3):
    nc = bacc.Bacc("TRN2", target_bir_lowering=False, debug=False)

    xa_d = nc.dram_tensor("xa", [128, 3, XR_H, XR_W], F8, kind="ExternalInput").ap()
    phi_d = nc.dram_tensor("phi", [128, 2, 3, ROWS, W], F8, kind="ExternalInput").ap()
    wc_d = nc.dram_tensor("wc", [128, NTK, 2, C_PHI], F8, kind="ExternalInput").ap()
    w1_d = nc.dram_tensor("w1", [128, 6, 2, HID], F8, kind="ExternalInput").ap()
    w2_d = nc.dram_tensor("w2", [128, 5, 2, HID], F8, kind="ExternalInput").ap()
    w3_d = nc.dram_tensor("w3", [128, 5, 2, 2 * C_LAT], F8, kind="ExternalInput").ap()
    bias_d = nc.dram_tensor("bias", [128, 21], F32, kind="ExternalInput").ap()
    lik_d = nc.dram_tensor("lik", [C_LAT, ROWS, W], F16, kind="ExternalOutput").ap()

    S = float(2.0 ** k3)
    CLAMP = float(0.11 * SQRT2 * S)

    with tile.TileContext(nc) as tc:
        with tc.tile_pool(name="const", bufs=1) as cpool, \
             tc.tile_pool(name="rp", bufs=5) as rpool, \
             tc.tile_pool(name="hp", bufs=2) as hpool, \
             tc.tile_pool(name="hcp", bufs=3) as hcpool, \
             tc.tile_pool(name="tp", bufs=8) as tpool, \
             tc.tile_pool(name="ps", bufs=8, space="PSUM") as pspool:

            wc_s = cpool.tile([128, NTK, 2, C_PHI], F8, tag="wc")
            w1_s = cpool.tile([128, 6, 2, HID], F8, tag="w1")
            w2_s = cpool.tile([128, 5, 2, HID], F8, tag="w2")
            w3_s = cpool.tile([128, 5, 2, 2 * C_LAT], F8, tag="w3")
            bs_s = cpool.tile([128, 21], F32, tag="bs")
            bc_s = bs_s[:, 0:3]
            b1_s = bs_s[:, 3:8]
            b2_s = bs_s[:, 8:13]
            b3_s = bs_s[:, 13:21]

            st = {}  # per-chunk tile state

            # split-op engine rotation (13 splits/chunk): slots 0-1 of each
            # mlp stage stay on DVE (they gate the next stage's first DRs);
            # Pool takes late slots only
            v, g = nc.vector, nc.gpsimd
            hi_cycle = [v, g, v,  v, v, v, g, v,  v, v, v, g, v]
            lo_cycle = [v, g, v,  v, v, g, v, g,  v, v, g, v, g]

            def split(h16v, hhv, hlv, idx):
                hi_cycle[idx].tensor_copy(hhv, h16v)
                lo_cycle[idx].tensor_tensor(hlv, h16v, hhv, ALU.subtract)

            def evac(p_v, h16v, bcol, kk, m, ci, lbl, prelu):
                """psum -> fp16 h. Wide (drain) chunks spread the work:
                ACT for m 0/3, DVE/Pool two-op affine(+leaky) otherwise."""
                func = AF.Prelu if prelu else AF.Identity
                sc_ = float(2.0 ** -kk)
                if ci < NCH - 2 or m in (0, 2, 3):
                    nc.scalar.activation(h16v, p_v, func, bias=bcol,
                                         scale=sc_,
                                         **({"alpha": 0.01} if prelu else {}))
                    return
                # DVE reads PSUM for the affine; Pool (no PSUM access) does
                # the SBUF-to-SBUF leaky
                if not prelu:
                    v.tensor_scalar(h16v, p_v, sc_, bcol, ALU.mult, ALU.add)
                    return
                aff = tpool.tile([128, 512], F16, tag="ev",
                                 name=f"af{lbl}_{m}_{ci}")
                n = h16v.shape[-1]
                v.tensor_scalar(aff[:, :n], p_v, sc_, bcol, ALU.mult, ALU.add)
                v.scalar_tensor_tensor(h16v, aff[:, :n], 0.01, aff[:, :n],
                                       ALU.mult, ALU.max)

            pf = {}

            def prefetch(ci):
                """Issue chunk ci's input DMAs one iteration ahead."""
                y0, rows = CHUNKS[ci]
                nr = rows + 2
                XA = rpool.tile([128, 3, 6, W + 4], F8, tag="XA", name=f"XA{ci}")
                if ci == 0:
                    # stage the first matmul's critical path in tiny DMAs
                    nc.sync.dma_start(wc_s[:, 0:1], wc_d[:, 0:1])
                    nc.sync.dma_start(XA[:, 0, 0:nr], xa_d[:, 0, y0:y0 + nr, 0:W + 4])
                    nc.sync.dma_start(wc_s[:, 1:3], wc_d[:, 1:3])
                    nc.sync.dma_start(XA[:, 1, 0:nr], xa_d[:, 1, y0:y0 + nr, 0:W + 4])
                    nc.sync.dma_start(XA[:, 2, 0:nr], xa_d[:, 2, y0:y0 + nr, 0:W + 4])
                    nc.sync.dma_start(wc_s[:, 3:9], wc_d[:, 3:9])
                elif nr == 6:
                    nc.sync.dma_start(XA[:], xa_d[:, :, y0:y0 + nr, 0:W + 4])
                else:
                    for pl in range(3):
                        nc.sync.dma_start(XA[:, pl, 0:nr],
                                          xa_d[:, pl, y0:y0 + nr, 0:W + 4])
                HC = hcpool.tile([128, 2, 6, rows, W], F8, tag="HC", name=f"HC{ci}")
                nc.sync.dma_start(HC[:, 0, 0:3], phi_d[:, 0, :, y0:y0 + rows, :])
                nc.sync.dma_start(HC[:, 1, 0:3], phi_d[:, 1, :, y0:y0 + rows, :])
                if ci == 0:
                    nc.sync.dma_start(wc_s[:, 9:NTK], wc_d[:, 9:NTK])
                    nc.sync.dma_start(bs_s[:], bias_d)
                pf[ci] = (XA, HC)

            def conv_emit(ci):
                y0, rows = CHUNKS[ci]
                N = rows * 128
                XA, HC = pf.pop(ci)

                pc = [pspool.tile([128, 512], F32, tag="ps", name=f"pc{m}_{ci}")
                      for m in range(3)]
                for t in range(NTK):
                    if t < 12:
                        dy, dx = TAPS[t]
                        src = XA[:, 0]
                    elif t < 17:
                        dy, dx = TAPS[t - 12]
                        src = XA[:, 1]
                    else:
                        dy, dx = TAPS[10]
                        src = XA[:, 2]
                    win = src[:, 2 + dy:2 + rows + dy, 2 + dx:2 + dx + W]
                    winb = win.unsqueeze(1).broadcast_to([128, 2, rows, W])
                    for m in range(3):
                        ms = slice(m * 128, (m + 1) * 128)
                        nc.tensor.matmul(pc[m][:, :N], wc_s[:, t, :, ms], winb,
                                         start=(t == 0), stop=(t == NTK - 1),
                                         perf_mode=DR)
                hc16 = hpool.tile([128, 3, 512], F16, tag="hc16")
                for m in range(3):
                    evac(pc[m][:, :N], hc16[:, m, :N], bc_s[:, m:m + 1],
                         kc, m, ci, "c", False)
                    split(hc16[:, m, :N],
                          HC[:, 0, 3 + m].rearrange("p r w -> p (r w)"),
                          HC[:, 1, 3 + m].rearrange("p r w -> p (r w)"), m)
                st[ci] = dict(XA=XA, HC=HC, rows=rows, y0=y0, N=N)

            def _5k_seq(Wp, Ah, Al, ms, n):
                """8-DR term sequence for one m-group, ordered by how late its
                input split lands. Wp layout [128, ktile, 2(hi/lo), D]."""
                a4h = Ah[:, 4, :n].unsqueeze(1).broadcast_to([128, 2, n])
                a4l = Al[:, 4, :n].unsqueeze(1).broadcast_to([128, 2, n])
                return [
                    (Wp[:, 0:2, 0, ms], Ah[:, 0:2, :n]),
                    (Wp[:, 0:2, 1, ms], Ah[:, 0:2, :n]),
                    (Wp[:, 0:2, 0, ms], Al[:, 0:2, :n]),
                    (Wp[:, 2:4, 0, ms], Ah[:, 2:4, :n]),
                    (Wp[:, 2:4, 1, ms], Ah[:, 2:4, :n]),
                    (Wp[:, 2:4, 0, ms], Al[:, 2:4, :n]),
                    (Wp[:, 4, :, ms], a4h),
                    (Wp[:, 4, :, ms], a4l),
                ]

            def mlp_5k(psum_t, Wp, Ah, Al, ms, n):
                """group-serial: all 8 DRs of one m-group back-to-back."""
                seq = _5k_seq(Wp, Ah, Al, ms, n)
                for i, (wv, av) in enumerate(seq):
                    nc.tensor.matmul(psum_t, wv, av, start=(i == 0),
                                     stop=(i == len(seq) - 1), perf_mode=DR)

            def mlp_5k_phased(psums, Wp, Ah, Al, n):
                """phase-major across all m-groups: every group's term-j DRs
                issue before any group's term-j+1, so split production stays
                ahead of consumption."""
                seqs = [_5k_seq(Wp, Ah, Al, slice(m * 128, (m + 1) * 128), n)
                        for m in range(len(psums))]
                for ph in range(8):
                    for m, pt in enumerate(psums):
                        wv, av = seqs[m][ph]
                        nc.tensor.matmul(pt, wv, av, start=(ph == 0),
                                         stop=(ph == 7), perf_mode=DR)

            def mlp1_emit(ci):
                s = st[ci]
                N = s["N"]
                HCv = s["HC"]
                Ah, Al = HCv[:, 0], HCv[:, 1]
                p1 = [pspool.tile([128, 512], F32, tag="ps", name=f"p1_{m}_{ci}")
                      for m in range(5)]
                for m in range(5):
                    ms = slice(m * 128, (m + 1) * 128)
                    seq = []
                    for j in range(3):
                        seq += [(0, Ah, j), (1, Ah, j), (0, Al, j)]
                    for i, (hl, hs, j) in enumerate(seq):
                        nc.tensor.matmul(p1[m][:, :N],
                                         w1_s[:, 2 * j:2 * j + 2, hl, ms],
                                         hs[:, 2 * j:2 * j + 2],
                                         start=(i == 0), stop=(i == len(seq) - 1),
                                         perf_mode=DR)
                h1_16 = hpool.tile([128, 5, 512], F16, tag="h1_16")
                H1h = hpool.tile([128, 5, 512], F8, tag="H1h")
                H1l = hpool.tile([128, 5, 512], F8, tag="H1l")
                for m in range(5):
                    evac(p1[m][:, :N], h1_16[:, m, :N], b1_s[:, m:m + 1],
                         k1, m, ci, "1", True)
                    split(h1_16[:, m, :N], H1h[:, m, :N], H1l[:, m, :N], 3 + m)
                if ci == 0:
                    nc.sync.dma_start(w2_s[:, 0:2], w2_d[:, 0:2])
                    nc.sync.dma_start(w2_s[:, 2:5], w2_d[:, 2:5])
                s["H1h"], s["H1l"] = H1h, H1l

            def mlp2_emit(ci):
                s = st[ci]
                N = s["N"]
                H1h, H1l = s["H1h"], s["H1l"]
                p2 = [pspool.tile([128, 512], F32, tag="ps", name=f"p2_{m}_{ci}")
                      for m in range(5)]
                for m in range(5):
                    ms = slice(m * 128, (m + 1) * 128)
                    mlp_5k(p2[m][:, :N], w2_s, H1h, H1l, ms, N)
                h2_16 = hpool.tile([128, 5, 512], F16, tag="h2_16")
                H2h = hpool.tile([128, 5, 512], F8, tag="H2h")
                H2l = hpool.tile([128, 5, 512], F8, tag="H2l")
                for m in range(5):
                    evac(p2[m][:, :N], h2_16[:, m, :N], b2_s[:, m:m + 1],
                         k2, m, ci, "2", True)
                    split(h2_16[:, m, :N], H2h[:, m, :N], H2l[:, m, :N], 8 + m)
                if ci == 0:
                    nc.sync.dma_start(w3_s[:], w3_d)
                s["H2h"], s["H2l"] = H2h, H2l  # noqa

            def lik_emit(ci, s, g, P, pm, psc, Rg, cpc, cmc, bsc, bsc_base):
                """Likelihood chain for one channel group. `psc` may sit at a
                nonzero PSUM base partition; the ACT Abs evac realigns it to
                base 0 (PSUM in + SB out cross-base is legal)."""
                rows, y0, N = s["rows"], s["y0"], s["N"]
                tg = f"t{g}"
                Rc = Rg[0:P, 2:2 + rows, 2:2 + W]
                # scale chain first: abs -> max -> recip (off mean critical path)
                sabs = tpool.tile([P, 512], F32, tag=tg, name=f"sa{g}_{ci}")
                nc.scalar.activation(sabs[:, :N], psc[:, :N], AF.Abs,
                                     bias=b3_s[bsc_base:bsc_base + P,
                                               bsc:bsc + 1])
                sc = tpool.tile([P, 512], F32, tag=tg, name=f"sc{g}_{ci}")
                nc.gpsimd.tensor_scalar_max(sc[:, :N], sabs[:, :N], CLAMP)
                rq = tpool.tile([P, 512], F32, tag=tg, name=f"rq{g}_{ci}")
                nc.vector.reciprocal_approx_fast(out=rq[:, :N], in_=sc[:, :N])
                tt = tpool.tile([P, 512], F16, tag=tg, name=f"tt{g}_{ci}")
                nc.vector.scalar_tensor_tensor(
                    tt[:, :N], Rc, -S, pm[:, :N], ALU.mult, ALU.add)
                # [em | ep] packed so one Erf covers both halves
                E = tpool.tile([P, 2, 512], F16, tag=tg, name=f"E{g}_{ci}")
                nc.vector.scalar_tensor_tensor(
                    E[:, 0, :N], tt[:, :N], b3_s[0:P, cpc:cpc + 1], rq[:, :N],
                    ALU.add, ALU.mult)
                nc.vector.scalar_tensor_tensor(
                    E[:, 1, :N], tt[:, :N], b3_s[0:P, cmc:cmc + 1], rq[:, :N],
                    ALU.add, ALU.mult)
                E2 = tpool.tile([P, 2, 512], F16, tag=tg, name=f"F{g}_{ci}")
                nc.scalar.activation(E2[:, :, :N], E[:, :, :N], AF.Erf)
                dd = tpool.tile([P, 512], F16, tag=tg, name=f"dd{g}_{ci}")
                ddeng = nc.vector if ci >= NCH - 1 else nc.gpsimd
                ddeng.tensor_tensor(dd[:, :N], E2[:, 0, :N], E2[:, 1, :N],
                                    ALU.subtract)
                ch0 = 0 if g == 0 else 128
                nc.sync.dma_start(lik_d[ch0:ch0 + P, y0:y0 + rows, :],
                                  dd[:, :N])

            def mlp3_emit(ci):
                s = st.pop(ci)
                N = s["N"]
                H2h, H2l, XA = s["H2h"], s["H2l"], s["XA"]
                # 3 M=128 groups, packed [mean128:192|scale128:192],
                # [scale0:128], [mean0:128]; the mixed group goes first so its
                # (small) likelihood chain overlaps the remaining matmuls, and
                # the g0 scale chain overlaps the g0 mean matmuls.
                p3 = []
                for mi in range(3):
                    pt = pspool.tile([128, 512], F32, tag="ps", name=f"p3_{mi}_{ci}")
                    mlp_5k(pt[:, :N], w3_s, H2h, H2l,
                           slice(mi * 128, (mi + 1) * 128), N)
                    p3.append(pt)
                    if mi == 0:
                        lik_emit(ci, s, 1, 64, pt[0:64], pt[64:128], XA[:, 1],
                                 4, 5, 6, 64)
                lik_emit(ci, s, 0, 128, p3[2], p3[1], XA[:, 0], 0, 1, 2, 0)

            # depth-4 software pipeline. mlp3 leads each iteration so its
            # likelihood chain ops sit at the head of the engine queues (p3
            # PSUM banks free fast) and every 8-back psum-pool pairing lands
            # on an ACT-evac-freed tile.
            for i in range(NCH + 3):
                if i < NCH:
                    # steady state: m3 first (lik ops take queue priority,
                    # p3 banks free fast)
                    if 3 <= i:
                        mlp3_emit(i - 3)
                    if i == 0:
                        prefetch(0)
                    conv_emit(i)
                    if i + 1 < NCH:
                        prefetch(i + 1)
                    if i == 0:
                        # k-tiles 0:2 first: mlp1(0)'s first DRs need these
                        nc.sync.dma_start(w1_s[:, 0:2], w1_d[:, 0:2])
                        nc.sync.dma_start(w1_s[:, 2:6], w1_d[:, 2:6])
                    if 1 <= i:
                        mlp1_emit(i - 1)
                    if 2 <= i:
                        mlp2_emit(i - 2)
                else:
                    # drain: evacs gate the remaining PE work — emit them
                    # ahead of the likelihood chains in the engine FIFOs
                    if i <= NCH:
                        mlp1_emit(i - 1)
                    if i <= NCH + 1:
                        mlp2_emit(i - 2)
                    mlp3_emit(i - 3)

    nc.compile()
    return nc


def _wsplit(wt):
    """per-tensor 2^k scaling + e4m3 hi/lo split. Returns (hi, lo, k)."""
    k = int(np.floor(np.log2(E4MAX / np.abs(wt).max())))
    ws = (wt * (2.0 ** k)).astype(np.float32)
    hi = ws.astype(F8NP)
    lo = (ws - hi.astype(np.float32)).astype(F8NP)
    return hi, lo, k


def _host_pack(mask_w, mask_b, w1, b1, w2, b2, w3, b3):
    wc = np.empty((C_LAT, NT, C_PHI), np.float32)
    for t, (dy, dx) in enumerate(TAPS):
        wc[:, t, :] = mask_w[:, :, dy + 2, dx + 2].T
    wcp = np.empty((128, NTK, C_PHI), np.float32)
    wcp[:, :12] = wc[:128]
    for j in range(6):
        ta, tb = (j, 5 + j) if j < 5 else (10, 11)
        wcp[0:64, 12 + j] = wc[128:, ta]
        wcp[64:128, 12 + j] = wc[128:, tb]
    wch, wcl, kc = _wsplit(wcp)
    wc8 = np.ascontiguousarray(np.stack([wch, wcl], axis=2))

    # w1 rows reordered: slots 0-2 = phi (rows 384:768), 3-5 = conv (0:384)
    w1r = np.concatenate([w1[C_PHI:], w1[:C_PHI]], axis=0)
    w1p = np.ascontiguousarray(w1r.reshape(6, 128, HID).transpose(1, 0, 2))
    w1h, w1l, k1 = _wsplit(w1p)
    w1pk = np.ascontiguousarray(np.stack([w1h, w1l], axis=2))

    w2p = np.ascontiguousarray(w2.reshape(5, 128, HID).transpose(1, 0, 2))
    w2h, w2l, k2 = _wsplit(w2p)
    w2pk = np.ascontiguousarray(np.stack([w2h, w2l], axis=2))

    w3m = w3.copy()
    w3m[:, C_LAT:] *= SQRT2
    # column order = [mean128:192 | scale128:192] [scale0:128] [mean0:128]
    # so mlp3 runs as 3 full-width M=128 groups (mixed group first)
    perm = np.concatenate([np.arange(128, 192), np.arange(320, 384),
                           np.arange(192, 320), np.arange(0, 128)])
    w3m = np.ascontiguousarray(w3m[:, perm])
    w3p = np.ascontiguousarray(w3m.reshape(5, 128, 2 * C_LAT).transpose(1, 0, 2))
    w3h, w3l, k3 = _wsplit(w3p)
    w3pk = np.ascontiguousarray(np.stack([w3h, w3l], axis=2))

    S = 2.0 ** k3
    b3pk = np.zeros((128, 8), np.float32)
    b3pk[:, 0] = S * (b3[0:128] + 0.5)
    b3pk[:, 1] = S * (b3[0:128] - 0.5)
    b3pk[:, 2] = S * SQRT2 * b3[192:320]
    b3pk[:64, 4] = S * (b3[128:192] + 0.5)
    b3pk[:64, 5] = S * (b3[128:192] - 0.5)
    # scale bias for ch 128:192 duplicated at both partition halves so the
    # ACT Abs bias is right under either base-alignment convention
    b3pk[:64, 6] = S * SQRT2 * b3[320:384]
    b3pk[64:, 6] = S * SQRT2 * b3[320:384]

    bias_pk = np.ascontiguousarray(np.concatenate(
        [mask_b.reshape(3, 128).T, b1.reshape(5, 128).T,
         b2.reshape(5, 128).T, b3pk], axis=1)).astype(np.float32)

    weights = {
        "wc": wc8,
        "w1": w1pk,
        "w2": w2pk,
        "w3": w3pk,
        "bias": bias_pk,
    }
    return weights, (kc, k1, k2, k3)


def kernel(x, phi, mask_w, mask_b, w1, b1, w2, b2, w3, b3):
    global LAST_RESULT
    x = np.asarray(x, dtype=np.float32)
    phi = np.asarray(phi, dtype=np.float32)
    weights, ks = _host_pack(
        np.asarray(mask_w, np.float32), np.asarray(mask_b, np.float32),
        np.asarray(w1, np.float32), np.asarray(b1, np.float32),
        np.asarray(w2, np.float32), np.asarray(b2, np.float32),
        np.asarray(w3, np.float32), np.asarray(b3, np.float32))

    R = np.round(x)
    R8 = R.astype(F8NP)
    phih = phi.astype(F8NP)
    phil = (phi - phih.astype(np.float32)).astype(F8NP)

    key = ("nc",) + ks
    if key not in _CACHE:
        _CACHE[key] = _build(*ks)
        _CACHE["nc"] = _CACHE[key]
    nc = _CACHE[key]

    in_maps = []
    for c in range(N_CORES):
        b, r0 = c // 2, (c % 2) * ROWS
        xr_c = np.zeros((C_LAT, XR_H, XR_W), F8NP)
        lo = max(r0 - 2, 0)
        hi = min(r0 + ROWS + 1, H)
        xr_c[:, 2 - (r0 - lo):2 - (r0 - lo) + (hi - lo), 2:2 + W] = R8[b, :, lo:hi, :]
        # 3 planes: [ch0:128] | [ch128:192; +1row] | [ch128:192; +1col]
        xa_c = np.zeros((128, 3, XR_H, XR_W), F8NP)
        xa_c[:, 0] = xr_c[0:128]
        xa_c[0:64, 1] = xr_c[128:192]
        xa_c[64:128, 1, 0:XR_H - 1] = xr_c[128:192, 1:XR_H]
        xa_c[0:64, 2] = xr_c[128:192]
        xa_c[64:128, 2, :, 0:XR_W - 1] = xr_c[128:192, :, 1:XR_W]
        # phi packed [128, 2, 3, ROWS, W]: hi/lo, partition-major k-tiles
        ph_c = np.ascontiguousarray(
            phih[b, :, r0:r0 + ROWS, :].reshape(3, 128, ROWS, W)
            .transpose(1, 0, 2, 3))
        pl_c = np.ascontiguousarray(
            phil[b, :, r0:r0 + ROWS, :].reshape(3, 128, ROWS, W)
            .transpose(1, 0, 2, 3))
        phi_c = np.ascontiguousarray(np.stack([ph_c, pl_c], axis=1))
        in_maps.append({"xa": xa_c, "phi": phi_c, **weights})

    res = run_bass_kernel_spmd(nc, in_maps, core_ids=list(range(N_CORES)),
                               trace=TRACE)
    LAST_RESULT = res

    lik = np.empty((B, C_LAT, H, W), np.float32)
    for c in range(N_CORES):
        b, r0 = c // 2, (c % 2) * ROWS
        lik[b, :, r0:r0 + ROWS, :] = \
            np.asarray(res.results[c]["lik"], np.float32) * 0.5
    return R, lik
